# revision 8
# baseline (speedup 1.0000x reference)
"""Bahdanau-attention kernel for Trainium2 (8 NeuronCores, Bass/Tile).

Computation (reference, fp32):
    Wh  = hidden @ W_w.T + W_b                      # [B, H]
    Ue  = einsum('bse,he->bsh', enc^T, U_w) + U_b   # [B, S, H]
    en  = tanh(Wh[:,None,:] + Ue) @ v_w[0]          # [B, S]
    out = softmax(where(mask, -1e10, en), axis=1)

Strategy
- Data-parallel over batch: 8 batches per core, weights replicated.
- Masked positions contribute exactly 0 to the softmax (exp(-1e10) = 0
  in fp32), so the host packs only the unmasked s-columns per batch row
  and scatters results back; the device computes energies only for
  packed columns. This is exact, not an approximation.
- Sorted-slot packing: the 64 rows are sorted by unmasked count and
  rank-grouped into 8 slots (one row per core per slot), so each slot's
  padded width is the max of 8 *similar* counts instead of the global
  max. Fully-masked rows are uniform 1/S by definition, fixed on host.
- Main matmul out[h, s] = U_w.T-chunk (stationary) x enc-chunk (moving)
  in bf16 with fp32 PSUM accumulation; 16 k-chunks of 128 accumulate in
  one PSUM bank per (slot, h-chunk).
- All DMA is laid out host-side to be per-partition contiguous so each
  logical transfer is a single large descriptor stream (fast ramp).
- A short burst of dummy matmuls on scratch SBUF warms the PE HAM clock
  gate (1.2 -> 2.4 GHz) during the initial DMA ramp, so real matmuls
  run at full clock from the start.
- Wh + W_b + U_b is folded into the tanh as a per-partition ACT bias;
  the Wh chains interleave with slot 0's main blocks so PE work paces
  the startup DMA stream.
- The v-projection runs on the (otherwise idle) Vector engine: each
  tanh tile is scaled by its per-partition v chunk and accumulated over
  h-chunks in SBUF; a single ones-vector M=1 matmul per slot does the
  final partition reduce.  The LAST slot instead reduces via direct
  lhsT=v-chunk matmuls accumulating in PSUM, which removes the vector
  chain from the kernel's tail critical path.
- The device applies exp per packed row; the softmax normalization
  (sum + divide over each row's unmasked entries) happens in the host
  unpack loop, removing the reciprocal/broadcast chain from the tail.
"""

import numpy as np
import ml_dtypes

B, S, H, E = 64, 512, 1024, 2048
NCORES = 8
BL = B // NCORES          # batches (slots) per core
HC = H // 128             # h chunks
EC = E // 128             # e (contraction) chunks
KC = H // 128             # k chunks for the Wh matmul

bf16 = ml_dtypes.bfloat16

_CACHE = {}

N_WARM = 16               # dummy warm-up matmuls (N=256) during DMA ramp
N_WARM2 = 10              # extra fine-grain warm-up matmuls (N=128)


def _build_nc(Ws):
    """Per-core program; Ws = tuple of 8 slot widths (mult of 4, <=512),
    in processing order (descending)."""
    import concourse.mybir as mybir
    import concourse.tile as tile
    from concourse import bacc

    F32 = mybir.dt.float32
    BF = mybir.dt.bfloat16
    AF = mybir.ActivationFunctionType

    Ws = list(Ws)
    TOT = sum(Ws)
    off = [0]
    for w in Ws:
        off.append(off[-1] + w)

    nc = bacc.Bacc(num_swdge_queues=4)
    # enc packed: [p, b, ec, s]  (per-partition contiguous per slot)
    enc_t = nc.declare_dram_parameter("enc_t", [128, EC * TOT], BF, isOutput=False)
    # U_w.T pre-chunked: [p(=e%128), hc, ec, v(=h%128)]
    uwT = nc.declare_dram_parameter("uwT", [128, HC * EC * 128], BF, isOutput=False)
    # W_w.T re-chunked: [p(=k%128), hc, kc, v(=h%128)]
    wwT = nc.declare_dram_parameter("wwT", [128, HC * KC * 128], BF, isOutput=False)
    hidT = nc.declare_dram_parameter("hidT", [128, KC * BL], BF, isOutput=False)
    vt = nc.declare_dram_parameter("vt", [128, HC], BF, isOutput=False)
    bc = nc.declare_dram_parameter("bc", [128, HC], F32, isOutput=False)
    out_d = nc.declare_dram_parameter("out", [1, TOT], F32, isOutput=True)

    def enc_base(b, ec):
        return EC * off[b] + ec * Ws[b]

    with tile.TileContext(nc) as tc:
        with (
            tc.tile_pool(name="const", bufs=1) as cst,
            tc.tile_pool(name="wpool", bufs=1) as wp,
            tc.tile_pool(name="thp", bufs=6) as thp,
            tc.tile_pool(name="accp", bufs=3) as accp,
            tc.tile_pool(name="pup", bufs=4, space="PSUM") as pup,
            tc.tile_pool(name="pep", bufs=2, space="PSUM") as pep,
            tc.tile_pool(name="pwp", bufs=1, space="PSUM") as pwp,
            tc.tile_pool(name="pwarm", bufs=1, space="PSUM") as pwarm,
        ):
            # ---- PE warm-up on scratch data (HAM clock-gate release) ---
            warm_sb = cst.tile([128, 384], BF, tag="warm")
            nc.vector.memset(warm_sb[:], 1.0)
            warm_ps = pwarm.tile([128, 256], F32, tag="wps")
            for _ in range(N_WARM):
                nc.tensor.matmul(
                    warm_ps[:], lhsT=warm_sb[:, 0:128], rhs=warm_sb[:, 128:384],
                    start=True, stop=True,
                )
            for _ in range(N_WARM2):
                nc.tensor.matmul(
                    warm_ps[:, 0:128], lhsT=warm_sb[:, 0:128],
                    rhs=warm_sb[:, 128:256], start=True, stop=True,
                )

            # ---- constants / weights ------------------------------------
            # DMA order = need order: the Wh prologue needs hid+ww first;
            # main block (0, hc) needs uw[hc] + enc slot 0.
            hid_sb = cst.tile([128, KC * BL], BF, tag="hid")
            ww_sb = wp.tile([128, HC * KC * 128], BF, tag="ww")
            uw_sb = wp.tile([128, HC * EC * 128], BF, tag="uw")
            enc_sb = wp.tile([128, EC * TOT], BF, tag="enc")
            bc_sb = cst.tile([128, HC], F32, tag="bc")
            vt_sb = cst.tile([128, HC], BF, tag="vt")

            WWC = KC * 128   # ww cols per hc
            UWC = EC * 128   # uw cols per hc

            def ww_sl(hc):
                return slice(hc * WWC, (hc + 1) * WWC)

            def uw_sl(hc):
                return slice(hc * UWC, (hc + 1) * UWC)

            # q1 = sync, q2 = gpsimd.  Order = PE need order.  The Wh
            # prologue (all 8 chains, run before the main loop) needs
            # hid+ww only, so those go first and pace the PE while the
            # bigger uw/enc stream lands behind them.
            nc.sync.dma_start(hid_sb[:], hidT[:])
            nc.gpsimd.dma_start(bc_sb[:], bc[:])
            nc.gpsimd.dma_start(vt_sb[:], vt[:])
            for hc in range(HC):
                eng = (nc.sync, nc.gpsimd)[hc % 2]
                eng.dma_start(ww_sb[:, ww_sl(hc)], wwT[:, ww_sl(hc)])
            UH = UWC // 2
            nc.sync.dma_start(uw_sb[:, 0:UH], uwT[:, 0:UH])
            nc.gpsimd.dma_start(uw_sb[:, UH:UWC], uwT[:, UH:UWC])
            # slot-0/1 enc in 4 ec-groups, alternating queues (fine grain
            # so the first chains can start on partial data)
            EG = EC // 4
            for b01 in range(2):
                if b01 == 1:
                    a = 1 * UWC
                    nc.sync.dma_start(uw_sb[:, a:a + UH], uwT[:, a:a + UH])
                    nc.gpsimd.dma_start(
                        uw_sb[:, a + UH:a + UWC], uwT[:, a + UH:a + UWC])
                for g in range(4):
                    a = enc_base(b01, g * EG)
                    bnd = (enc_base(b01, (g + 1) * EG)
                           if g < 3 else enc_base(b01 + 1, 0))
                    eng = (nc.sync, nc.gpsimd)[g % 2]
                    eng.dma_start(enc_sb[:, a:bnd], enc_t[:, a:bnd])
            # remaining uw chunks, then remaining enc slots
            for hc in range(2, HC):
                a = hc * UWC
                eng = (nc.sync, nc.gpsimd)[hc % 2]
                eng2 = (nc.gpsimd, nc.sync)[hc % 2]
                eng.dma_start(uw_sb[:, a:a + UH], uwT[:, a:a + UH])
                eng2.dma_start(uw_sb[:, a + UH:a + UWC], uwT[:, a + UH:a + UWC])
            for b in range(2, BL):
                a = enc_base(b, 0)
                m = enc_base(b, EC // 2)
                bnd = enc_base(b + 1, 0) if b + 1 < BL else EC * TOT
                eng = (nc.sync, nc.gpsimd)[b % 2]
                eng2 = (nc.gpsimd, nc.sync)[b % 2]
                eng.dma_start(enc_sb[:, a:m], enc_t[:, a:m])
                eng2.dma_start(enc_sb[:, m:bnd], enc_t[:, m:bnd])

            bias_sb = cst.tile([128, HC * BL], F32, tag="bias")
            res_sb = cst.tile([1, TOT], F32, tag="res")
            ones_sb = cst.tile([128, 1], BF, tag="ones")
            nc.vector.memset(ones_sb[:], 1.0)

            # ---- Wh prologue: all 8 chains upfront ----------------------
            # These need only hid+ww (~2.3 MB, first in DMA order), so
            # they keep the PE warm while the uw/enc stream lands.
            for hc in range(HC):
                pw = pwp.tile([128, BL], F32, tag="pw")
                for kc in range(KC):
                    nc.tensor.matmul(
                        pw[:],
                        lhsT=ww_sb[:, hc * WWC + kc * 128:
                                   hc * WWC + (kc + 1) * 128],
                        rhs=hid_sb[:, kc * BL:(kc + 1) * BL],
                        start=(kc == 0),
                        stop=(kc == KC - 1),
                    )
                nc.vector.tensor_tensor(
                    bias_sb[:, hc * BL:(hc + 1) * BL], pw[:],
                    bc_sb[:, hc:hc + 1].to_broadcast([128, BL]),
                    mybir.AluOpType.add,
                )

            # ---- main loop over slots -----------------------------------
            for b in range(BL):
                W = Ws[b]
                last = b == BL - 1
                pe_ = pep.tile([1, W], F32, tag="pe")
                if not last:
                    acc = accp.tile([128, W], F32, tag="acc")
                for hc in range(HC):
                    pu = pup.tile([128, W], F32, tag="pu")
                    for ec in range(EC):
                        a = enc_base(b, ec)
                        nc.tensor.matmul(
                            pu[:],
                            lhsT=uw_sb[:, hc * UWC + ec * 128:
                                       hc * UWC + (ec + 1) * 128],
                            rhs=enc_sb[:, a:a + W],
                            start=(ec == 0),
                            stop=(ec == EC - 1),
                        )
                    th = thp.tile([128, W], BF, tag="th")
                    nc.scalar.activation(
                        th[:], pu[:], AF.Tanh,
                        bias=bias_sb[:, hc * BL + b:hc * BL + b + 1],
                    )
                    if last:
                        # direct v-chunk reduce on PE: shortest tail path
                        nc.tensor.matmul(
                            pe_[0:1, :], lhsT=vt_sb[:, hc:hc + 1], rhs=th[:],
                            start=(hc == 0), stop=(hc == HC - 1),
                        )
                    else:
                        # v-weighting on the (otherwise idle) Vector engine
                        vcol = vt_sb[:, hc:hc + 1].to_broadcast([128, W])
                        if hc == 0:
                            nc.vector.tensor_tensor(
                                acc[:], th[:], vcol, mybir.AluOpType.mult)
                        else:
                            tmp = thp.tile([128, W], F32, tag="tmp")
                            nc.vector.tensor_tensor(
                                tmp[:], th[:], vcol, mybir.AluOpType.mult)
                            nc.vector.tensor_add(acc[:], acc[:], tmp[:])
                if not last:
                    # single partition-reduce matmul replaces the 8 v-dots
                    accb = thp.tile([128, W], BF, tag="accb")
                    nc.vector.tensor_copy(accb[:], acc[:])
                    nc.tensor.matmul(
                        pe_[0:1, :], lhsT=ones_sb[:, 0:1], rhs=accb[:],
                        start=True, stop=True,
                    )

                # ---- exp over packed columns; normalization on host -----
                sl = slice(off[b], off[b] + W)
                nc.scalar.activation(res_sb[0:1, sl], pe_[0:1, :], AF.Exp)
                nc.sync.dma_start(out_d[0:1, sl], res_sb[0:1, sl])

    nc.finalize()
    return nc


def _prep_inputs(hidden, encoder_outputs, mask, W_w, W_b, U_w, U_b, v_w):
    enc_bf = encoder_outputs.astype(bf16)          # [S, B, E]
    # U_w.T [E, H] -> [p(=e%128), hc, ec, v(=h%128)]
    uwT_np = np.ascontiguousarray(U_w.T).astype(bf16)
    uwT_np = np.ascontiguousarray(
        uwT_np.reshape(EC, 128, HC, 128).transpose(1, 2, 0, 3)
    ).reshape(128, HC * EC * 128)
    wwT_np = np.ascontiguousarray(W_w.T).astype(bf16)
    wwT_np = np.ascontiguousarray(
        wwT_np.reshape(KC, 128, HC, 128).transpose(1, 2, 0, 3)
    ).reshape(128, HC * KC * 128)
    vt_np = np.ascontiguousarray(v_w[0].reshape(HC, 128).T).astype(bf16)
    bc_np = np.ascontiguousarray((W_b + U_b).reshape(HC, 128).T).astype(np.float32)

    idx_all = [np.nonzero(~mask[i])[0] for i in range(B)]
    counts = np.array([len(ix) for ix in idx_all])

    # sorted-slot packing: rank-group rows into 8 slots of 8 (one per core)
    order = np.argsort(-counts, kind="stable")
    rows = order.reshape(BL, NCORES)       # rows[b][c] = global row index
    Ws = tuple(int(max(4, 4 * -(-counts[rows[b]].max() // 4)))
               for b in range(BL))
    TOT = sum(Ws)
    off = np.concatenate([[0], np.cumsum(Ws)]).astype(int)

    in_maps = []
    for c in range(NCORES):
        crows = rows[:, c]                                       # slot -> row
        # enc for this core's rows: [E, BL, S]
        enc_c = np.ascontiguousarray(enc_bf[:, crows, :].transpose(2, 1, 0))
        enc_p = np.zeros((128, EC * TOT), bf16)
        for b in range(BL):
            ix = idx_all[crows[b]]
            cnt = len(ix)
            if cnt:
                # [E, cnt] -> [EC, 128, cnt] -> [128, EC, cnt]
                g = enc_c[:, b, ix].reshape(EC, 128, cnt).transpose(1, 0, 2)
                v = enc_p[:, EC * off[b]:EC * off[b + 1]].reshape(128, EC, Ws[b])
                v[:, :, :cnt] = g
        hid_c = hidden[crows].astype(bf16)                       # [BL, H]
        hidT_c = np.ascontiguousarray(
            hid_c.T.reshape(KC, 128, BL).transpose(1, 0, 2)
        ).reshape(128, KC * BL)
        in_maps.append({
            "enc_t": enc_p,
            "uwT": uwT_np,
            "wwT": wwT_np,
            "hidT": hidT_c,
            "vt": vt_np,
            "bc": bc_np,
        })
    return in_maps, Ws, rows, idx_all, counts


def _run(in_maps, Ws, trace=False):
    from concourse import bass_utils
    if Ws not in _CACHE:
        _CACHE[Ws] = _build_nc(Ws)
    nc = _CACHE[Ws]
    return bass_utils.run_bass_kernel_spmd(
        nc, in_maps, core_ids=list(range(NCORES)), trace=trace
    )


def kernel(hidden, encoder_outputs, mask, W_w, W_b, U_w, U_b, v_w,
           _trace=False, _return_bkr=False):
    hidden = np.asarray(hidden, dtype=np.float32)
    encoder_outputs = np.asarray(encoder_outputs, dtype=np.float32)
    mask = np.asarray(mask).astype(bool)
    W_w = np.asarray(W_w, dtype=np.float32)
    W_b = np.asarray(W_b, dtype=np.float32)
    U_w = np.asarray(U_w, dtype=np.float32)
    U_b = np.asarray(U_b, dtype=np.float32)
    v_w = np.asarray(v_w, dtype=np.float32)

    in_maps, Ws, rows, idx_all, counts = _prep_inputs(
        hidden, encoder_outputs, mask, W_w, W_b, U_w, U_b, v_w)
    bkr = _run(in_maps, Ws, trace=_trace)

    offs = np.concatenate([[0], np.cumsum(Ws)]).astype(int)
    out = np.zeros((B, S), np.float32)
    for c in range(NCORES):
        dev = bkr.results[c]["out"].reshape(-1)
        for b in range(BL):
            i = rows[b, c]
            cnt = counts[i]
            if cnt:
                e = dev[offs[b]:offs[b] + cnt]
                out[i, idx_all[i]] = e / e.sum()
            else:
                # fully-masked row: softmax over all -1e10 is uniform
                out[i, :] = np.float32(1.0 / S)
    if _return_bkr:
        return out, bkr
    return out
